# revision 47
# baseline (speedup 1.0000x reference)
"""Trainium2 Bass kernel for nn_Attention_12369505813001.

Computes, per batch b:
    qw    = query @ W_in.T                      [T, H]
    score = qw @ enc.T                          [T, S]
    p     = softmax(mask(score), axis=S)
    c     = p @ enc                             [T, H]
    out   = tanh(concat(query, c) @ W_out.T + b_out)

Shapes: B=32, T=512, S=1024, H=1024, fp32. Data-parallel over B across
8 NeuronCores (4 batches/core); no collectives.

Layout strategy (per core): feature dim on partitions, T on the free
axis throughout, so the PE contraction dim always lands on partitions
and no on-device transposes are needed:
    step1  qw^T[o,t]    = W_inT-tiles(stat) @ q^T(moving)
    step2  score^T[s,t] = encT-tiles(stat)  @ qw^T(moving)
    softmax over s (partition+chunk axis): per-batch global max via
      free-axis max tree + GPSIMD partition all-reduce(max); exp on ACT
      with per-partition bias = additive length mask; denominator via a
      DVE chunk-sum tree + GPSIMD partition all-reduce(add); the
      normalization is folded into c as a broadcast mul.
    step4  c~^T[h,t]    = enc-tiles(stat)   @ e^T(moving)     fp16
    step5  out^T[o,t]   = tanh(WqT(stat) @ q^T + WcT(stat) @ cn + b)  fp16

Precision: the softmax path must be accurate — score noise of 3e-3 rms
already breaches the 2e-2 gate at near-tie softmax columns (f32r at
~11 mantissa bits was measured at 5.3e-2 absmax end-to-end — unusable)
— so steps 1-2 use a split scheme at 1.5 matmul passes: a main fp16
hi*hi pass plus ONE fp8e4m3 DoubleRow pass computing both cross terms
(lo*hi + hi*lo) at 0.5 cyc/row. Residuals lo = x - fp16(x) are
prescaled by 2^11 into fp8 range; the cross PSUM is folded back as
score = main + 2^-11 * cross on DVE. Splits of q, encT, W_in are
host-precomputed; qw is split on device. Steps 4-5 are insensitive and
run fully in fp16; the output is stored fp16 and upcast on host.
Because e is stored fp16 (subnormal floor ~6e-8), the softmax max MUST
exclude masked positions — a masked global max would flush every real
exp to zero and divide by a zero denominator — so the mask is folded
into the max tree (fused add+max against the per-partition mask
scalar).

Masked-length skipping: src_lengths ∈ [S/2, S], so on average 1/4 of
the S axis is fully masked. The host sorts all B batches by length
(descending) and assigns rank-group g (of 8) to per-core batch slot g;
the program is compiled with a per-slot s-chunk count
ks[g] = ceil(max length in group g / 128) and skips the masked
s-chunks entirely in step2, softmax, and step4. The compiled program
is cached per ks tuple (all cores share one program, so slot counts
must cover the group max — the sorted assignment minimizes the sum).

Schedule: software-pipelined across batches. Per iteration the PE runs
step1(b+1), step4(b), step2(b+1), step5(b): softmax(b)'s DVE/ACT chain
hides under step1(b+1), softmax(b+1) under step5(b)+step1(b+2), and
hoisting step2(b+1) before step5(b) keeps its ACT score-copies (which
gate the step2 PSUM rotation) ahead of step5's tanh+out-DMA block on
the serial ACT sequencer (an ACT-queue dma_start costs 667ns of
sequencer time). The PE p-state ramp (0.65/1.2GHz until ~3us of
continuous execution) is paid during the prologue DMA wait by warming
the PE with dummy matmuls on a zeroed tile. qw folds use a residual
identity (qwhi = fp16(main-PSUM); res = (main - qwhi) + 2^-11*cross)
at 2 DVE + 2 ACT + 1/2 GPSIMD ops per chunk, with the cross-dependent
tail deferred one chunk so it never blocks the next chunk's PSUM
eviction on the in-order DVE queue. DMA is dominated by a serial
descriptor-generation path, so the kernel uses few, large DMA
instructions with >=512B contiguous runs (sub-512B runs pay 2x DMA
latency: encT/enc tiles are loaded in 256-column pairs, wi8 in
k-halves with 1KB runs), emitted in exact consumption order; the
prologue interleaves wihi o-slices with the q halves so the six
look-ahead mains of step1(0) start at ~5us. Output DMAs issue from the
Activation queue (SP for the last batch, whose input stream is done);
the final output chunk is split into four quarters in distinct
buffers so most of its tanh->DMA chain overlaps remaining matmuls.
"""

from contextlib import ExitStack

import numpy as np
import ml_dtypes

import concourse.bass as bass
import concourse.bass_isa as bass_isa
import concourse.mybir as mybir
import concourse.tile as tile
from concourse import bacc
from concourse.bass_utils import run_bass_kernel_spmd

B, T, S, H = 32, 512, 1024, 1024
NCORES = 8
BPC = B // NCORES          # batches per core
HT = H // 128              # h/o chunk count
ST = S // 128              # s chunk count
P = 128

f32 = mybir.dt.float32
bf16 = mybir.dt.bfloat16
fp16 = mybir.dt.float16
fp8 = mybir.dt.float8e4
AF = mybir.ActivationFunctionType
ALU = mybir.AluOpType
DR = mybir.MatmulPerfMode.DoubleRow

MASKVAL = -1.0e38
RSC = 2048.0               # 2^11 residual prescale
RSCI = 1.0 / RSC

_nc_cache = []             # [0] = active compiled nc (test.py reads this)
_nc_by_ks = {}
LAST_RESULTS = None


def _build_nc(ks):
    """ks: per-slot s-chunk counts (len BPC, each in [1, ST])."""
    nc = bacc.Bacc("TRN2", target_bir_lowering=False, debug=False)

    # moving packs: [hi8, lo8'] pairs; stationary packs: [lo8', hi8]
    qhi = nc.dram_tensor("qhi", [BPC, H, T], fp16, kind="ExternalInput")
    q8 = nc.dram_tensor("q8", [BPC, 2, H, T], fp8, kind="ExternalInput")
    eThi = nc.dram_tensor("eThi", [BPC, H, S], fp16, kind="ExternalInput")
    eT8 = nc.dram_tensor("eT8", [BPC, 2, H, S], fp8, kind="ExternalInput")
    encf = nc.dram_tensor("encf", [BPC, S, H], fp16, kind="ExternalInput")
    maskc = nc.dram_tensor("maskc", [BPC, P, ST], f32, kind="ExternalInput")
    Wihi = nc.dram_tensor("Wihi", [H, H], fp16, kind="ExternalInput")  # [h,o]
    Wi8 = nc.dram_tensor("Wi8", [2, H, H], fp8, kind="ExternalInput")
    Wqf = nc.dram_tensor("Wqf", [H, H], fp16, kind="ExternalInput")
    Wcf = nc.dram_tensor("Wcf", [H, H], fp16, kind="ExternalInput")
    bo = nc.dram_tensor("bo", [P, HT], f32, kind="ExternalInput")
    outT = nc.dram_tensor("outT", [BPC, H, T], fp16, kind="ExternalOutput")

    def kpairs(b):                 # number of 256-col encT/enc pair tiles
        return (ks[b] + 1) // 2

    with tile.TileContext(nc) as tc, ExitStack() as ctx:
        wp = ctx.enter_context(tc.tile_pool(name="wp", bufs=1))
        pq = ctx.enter_context(tc.tile_pool(name="pq", bufs=2))
        pa = ctx.enter_context(tc.tile_pool(name="pa", bufs=2))   # qw
        pcs = ctx.enter_context(tc.tile_pool(name="pcs", bufs=1))  # score/cn
        sp = ctx.enter_context(tc.tile_pool(name="sp", bufs=1))
        fpt = ctx.enter_context(tc.tile_pool(name="fpt", bufs=2))  # fold tmps
        pe1 = ctx.enter_context(tc.tile_pool(name="pe1", bufs=1))
        etp = ctx.enter_context(tc.tile_pool(name="etp", bufs=3))
        et8p = ctx.enter_context(tc.tile_pool(name="et8p", bufs=1))
        enp = ctx.enter_context(tc.tile_pool(name="enp", bufs=2))
        otp = ctx.enter_context(tc.tile_pool(name="otp", bufs=1))
        psA = ctx.enter_context(tc.tile_pool(name="psA", bufs=3, space="PSUM"))
        psB = ctx.enter_context(tc.tile_pool(name="psB", bufs=3, space="PSUM"))
        psC = ctx.enter_context(tc.tile_pool(name="psC", bufs=2, space="PSUM"))

        # --- persistent weights (whole-tensor DMAs; o-contiguous runs) ---
        wihi = wp.tile([P, HT, H], fp16, name="wihi")
        wi8 = wp.tile([P, 2, HT, H], fp8, name="wi8")
        wqf = wp.tile([P, HT, H], fp16, name="wqf")
        wcf = wp.tile([P, HT, H], fp16, name="wcf")
        bo_sb = wp.tile([P, HT], f32)
        mask_sb = wp.tile([P, BPC, ST], f32)

        def load_q(b, split=False):
            # split=True: emit the fp16 halves only (prologue interleaving);
            # caller must emit_q8 separately
            th = pq.tile([P, HT, T], fp16, tag="qhi", name=f"qhi_{b}")
            nc.sync.dma_start(
                out=th[:, 0:4, :],
                in_=qhi[b, 0:512].rearrange("(k p) t -> p k t", p=P))
            nc.sync.dma_start(
                out=th[:, 4:8, :],
                in_=qhi[b, 512:1024].rearrange("(k p) t -> p k t", p=P))
            t8 = pq.tile([P, 2, HT, T], fp8, tag="q8", name=f"q8_{b}")
            if not split:
                emit_q8(b, t8)
            return th, t8

        def emit_q8(b, t8):
            nc.sync.dma_start(
                out=t8, in_=q8[b].rearrange("c (k p) t -> p c k t", p=P))

        def emit_et(b, j):
            # 256-col pair tile: 512B runs (1x DMA latency class)
            eh = etp.tile([P, HT, 256], fp16, tag="et", name=f"et_{b}_{j}")
            nc.sync.dma_start(
                out=eh,
                in_=eThi[b, :, 256 * j:256 * (j + 1)]
                .rearrange("(k p) s -> p k s", p=P))
            return eh

        e8ws = {}

        def load_et8w(b):
            # whole-batch fp8 cross pack, truncated at the slot's chunk
            # count: 1KB-class contiguous runs avoid the sub-512B
            # descriptor latency penalty of per-m tiles
            t = et8p.tile([P, 2, HT, S], fp8, tag="e8w", name=f"e8w_{b}")
            sl = 128 * ks[b]
            nc.sync.dma_start(
                out=t[:, :, :, 0:sl],
                in_=eT8[b, :, :, 0:sl].rearrange("c (k p) s -> p c k s", p=P))
            return t

        qs = {}
        qws = {}
        ets = {}

        def emit_main(b, m, pool=None):
            th = qs[b][0]
            msl = slice(128 * m, 128 * (m + 1))
            mp = (pool or psA).tile([P, T], f32,
                                    tag="qo" if pool is None else "sc",
                                    name=f"qwm_{b}_{m}")
            for k in range(HT):
                nc.tensor.matmul(mp, wihi[:, k, msl], th[:, k, :],
                                 start=(k == 0), stop=(k == HT - 1))
            return mp

        def emit_cross_chain(b, m, mp, qwhi, qw8, pool=None):
            t8 = qs[b][1]
            msl = slice(128 * m, 128 * (m + 1))
            cp = (pool or psA).tile([P, T], f32,
                                    tag="qo" if pool is None else "c",
                                    name=f"qwc_{b}_{m}")
            for k in range(HT):
                nc.tensor.matmul(cp, wi8[:, :, k, msl], t8[:, :, k, :],
                                 start=(k == 0), stop=(k == HT - 1),
                                 perf_mode=DR)
            # fold via the residual identity: qwhi = fp16(main) directly,
            # res = (main - qwhi) + 2^-11*cross, so the f32 qw is never
            # materialized (2 DVE + 3 ACT ops; a DVE op may read at most
            # one non-scalar PSUM input). The cross-dependent tail (stt +
            # qw8lo) is DEFERRED one chunk by the caller so it never sits
            # between sub(m) and sub(m+1) on the in-order DVE queue.
            tmp = fpt.tile([P, T], f32, tag="tmp")
            nc.scalar.copy(qwhi[:, m, :], mp)
            nc.vector.tensor_sub(tmp, mp, qwhi[:, m, :])
            return cp, tmp

        def fold_tail(qw8, m, cp, tmp):
            nc.vector.scalar_tensor_tensor(tmp, cp, RSCI, tmp,
                                           ALU.mult, ALU.add)
            nc.scalar.activation(qw8[:, 1, m, :], tmp, AF.Copy, scale=RSC)

        def step1(b):
            qwhi = pa.tile([P, HT, T], fp16, tag="A", name=f"qwhi_{b}")
            qw8 = pa.tile([P, 2, HT, T], fp8, tag="A8", name=f"qw8_{b}")
            if b == 0:
                # head: run six fp16 main chunks ahead (filling psA+psB
                # banks; psB's first slot holds the warm-up tile) so the PE
                # has work while the fp8 cross operands (wi8/q8) are still
                # arriving; crosses then go to psC (free until step4(0)).
                # Eviction legality: every pool re-alloc lands on a tile
                # whose fold was already emitted.
                mains = [emit_main(0, m) for m in range(3)]
                mains += [emit_main(0, m, pool=psB) for m in range(3, 6)]
                pend = None
                for m in range(HT):
                    nxt = emit_cross_chain(0, m, mains[m], qwhi, qw8,
                                           pool=psC)
                    if pend is not None:
                        fold_tail(qw8, m - 1, *pend)
                    if m % 2 == 1:
                        # qw8hi casts in pairs on GPSIMD: one launch
                        # overhead per two chunks keeps the Pool queue
                        # short (the smax reduce shares it)
                        nc.gpsimd.tensor_copy(qw8[:, 0, m - 1:m + 1, :],
                                              qwhi[:, m - 1:m + 1, :])
                    pend = nxt
                    if m + 6 < HT:
                        mains.append(emit_main(0, m + 6))
                fold_tail(qw8, HT - 1, *pend)
            else:
                # depth-2 look-ahead: the first cross would otherwise wait
                # on fold(m0)'s ACT/DVE chain to release its psA slot at
                # every batch start. Legal because the fold reads mp before
                # cp, so cross(m+1) evicting mp(m) only waits on fold ops
                # that themselves depend on nothing downstream.
                mains = [emit_main(b, 0), emit_main(b, 1)]
                pend = None
                for m in range(HT):
                    nxt = emit_cross_chain(b, m, mains[m], qwhi, qw8)
                    if pend is not None:
                        fold_tail(qw8, m - 1, *pend)
                    if m % 2 == 1:
                        nc.gpsimd.tensor_copy(qw8[:, 0, m - 1:m + 1, :],
                                              qwhi[:, m - 1:m + 1, :])
                    pend = nxt
                    if m + 2 < HT:
                        mains.append(emit_main(b, m + 2))
                fold_tail(qw8, HT - 1, *pend)
            return qwhi, qw8

        # --- PE warm-up ---
        # The cost of the PE p-state ramp (0.65/1.2 GHz until ~3us of
        # continuous execution) is paid during the prologue DMA wait by
        # feeding the PE dummy matmuls on a zeroed tile; the first real
        # matmul then issues at the full 2.4 GHz clock.
        warm = fpt.tile([P, T], f32, tag="tmp")
        nc.gpsimd.memset(warm[:, 0:128], 0.0)
        wps = psB.tile([P, 128], f32, tag="sc", name="warm_ps")
        for i in range(11):
            nc.tensor.matmul(wps, warm[:, 0:128], warm[:, 0:128],
                             start=(i == 0), stop=(i == 10))

        # --- prologue ---
        # DMA order = PE consumption order: wihi o-slices + fp16 q halves
        # feed the six look-ahead mains; fp8 packs (q8/wi8 k-halves, 1KB
        # runs) land while the mains run and gate the cross chains; encT
        # tiles for step2(0) follow; wqf/wcf (needed ~70us in) go last.
        nc.sync.dma_start(
            out=wihi[:, :, 0:256],
            in_=Wihi[:, 0:256].rearrange("(k p) o -> p k o", p=P))
        qs[0] = load_q(0, split=True)
        nc.sync.dma_start(
            out=wihi[:, :, 256:512],
            in_=Wihi[:, 256:512].rearrange("(k p) o -> p k o", p=P))
        nc.sync.dma_start(
            out=wihi[:, :, 512:1024],
            in_=Wihi[:, 512:1024].rearrange("(k p) o -> p k o", p=P))
        emit_q8(0, qs[0][1])
        for _c in range(2):
            nc.sync.dma_start(
                out=wi8[:, _c, 0:4, :],
                in_=Wi8[_c, 0:512, :].rearrange("(k p) o -> p k o", p=P))
        for _c in range(2):
            nc.sync.dma_start(
                out=wi8[:, _c, 4:8, :],
                in_=Wi8[_c, 512:1024, :].rearrange("(k p) o -> p k o", p=P))
        nc.sync.dma_start(out=mask_sb,
                          in_=maskc[:, :, :].rearrange("b p m -> p b m"))
        nc.sync.dma_start(out=bo_sb, in_=bo[:, :])
        e8ws[0] = load_et8w(0)
        ets[0] = [emit_et(0, j) for j in range(kpairs(0))]
        qws[0] = step1(0)

        def prefetch_et(b):
            e8ws[b] = load_et8w(b)
            ets[b] = [emit_et(b, j) for j in range(min(3, kpairs(b)))]

        def step2(b):
            if b not in ets:
                prefetch_et(b)
            # et top-up BEFORE the q load: the q load waits on a pq-pool
            # buffer freed only by step5(b-1) (which runs AFTER this
            # step2 on the PE), so anything queued behind it on the
            # serial SP DMA queue that THIS step2 consumes would deadlock
            for j in range(len(ets[b]), kpairs(b)):
                ets[b].append(emit_et(b, j))
            if b + 1 < BPC:
                qs[b + 1] = load_q(b + 1)
            if b == 0:
                # wqf/wcf (first needed at step5(0)) go behind q(1) on the
                # serial SP DMA queue
                nc.sync.dma_start(
                    out=wqf, in_=Wqf[:, :].rearrange("(k p) o -> p k o", p=P))
                nc.sync.dma_start(
                    out=wcf, in_=Wcf[:, :].rearrange("(k p) o -> p k o", p=P))
            score = pcs.tile([P, ST, T], f32, tag="B", name=f"score_{b}")
            smax = sp.tile([P, T], f32, tag="smax")
            qwhi, qw8 = qws[b]
            e8w = e8ws[b]
            for m in range(ks[b]):
                eh = ets[b][m // 2]
                ssl = slice(128 * (m % 2), 128 * (m % 2) + 128)
                msl2 = slice(128 * m, 128 * (m + 1))
                mp = psB.tile([P, T], f32, tag="sc", name=f"scm_{b}_{m}")
                for k in range(HT):
                    nc.tensor.matmul(mp, eh[:, k, ssl], qwhi[:, k, :],
                                     start=(k == 0), stop=(k == HT - 1))
                cp = psB.tile([P, T], f32, tag="sc", name=f"scc_{b}_{m}")
                for k in range(HT):
                    nc.tensor.matmul(cp, e8w[:, :, k, msl2], qw8[:, :, k, :],
                                     start=(k == 0), stop=(k == HT - 1),
                                     perf_mode=DR)
                nc.scalar.copy(score[:, m, :], mp)
                # max over UNMASKED positions only: with e stored in fp16,
                # a masked global max would flush every real exp below the
                # fp16 subnormal floor and zero the denominator. Emitted
                # BEFORE the cross fold: it reads only mp, and the next
                # chunk's cross evicting mp's PSUM slot waits on all mp
                # readers via the in-order DVE queue.
                if m == 0:
                    nc.vector.tensor_scalar_add(smax, mp,
                                                mask_sb[:, b, m:m + 1])
                else:
                    nc.vector.scalar_tensor_tensor(smax, mp,
                                                   mask_sb[:, b, m:m + 1],
                                                   smax, ALU.add, ALU.max)
                # fold cross back in-place: score = main + 2^-11 * cross
                nc.vector.scalar_tensor_tensor(score[:, m, :], cp, RSCI,
                                               score[:, m, :],
                                               ALU.mult, ALU.add)
            return score, smax

        def softmax_head(b, score, smax):
            # the smax partition-reduce is emitted BEFORE step1(b+1) so it
            # is not queued behind step1's slow GPSIMD qw8hi casts on the
            # in-order Pool queue (it gates the entire sub->exp chain)
            smax_all = sp.tile([P, T], f32, tag="smax_all")
            nc.gpsimd.partition_all_reduce(smax_all, smax, channels=P,
                                           reduce_op=bass_isa.ReduceOp.max)
            return score, smax_all

        def softmax(b, score, smax_all):
            e = pe1.tile([P, ST, T], fp16, tag="E", name=f"e_{b}")
            esum = sp.tile([P, T], f32, tag="smax")
            # interleave sub -> exp -> esum-accumulate per chunk so the
            # serial tail after the last exp is one add + reduce + recip
            for m in range(ks[b]):
                nc.vector.tensor_sub(score[:, m, :], score[:, m, :], smax_all)
                nc.scalar.activation(e[:, m, :], score[:, m, :], AF.Exp,
                                     bias=mask_sb[:, b, m:m + 1])
                if m == 1:
                    nc.vector.tensor_add(esum, e[:, 0, :], e[:, 1, :])
                elif m > 1:
                    nc.vector.tensor_add(esum, esum, e[:, m, :])
            esum_all = sp.tile([P, T], f32, tag="esum_all")
            nc.gpsimd.partition_all_reduce(esum_all, esum, channels=P,
                                           reduce_op=bass_isa.ReduceOp.add)
            rdenb = sp.tile([P, T], f32, tag="smax_all")
            nc.vector.reciprocal(rdenb, esum_all)
            return e, rdenb

        # Pipeline order per batch: step1(b+1) covers softmax(b)'s
        # DVE/ACT chain; step2(b+1) is hoisted BEFORE step5(b) so its
        # ACT score-copies (which gate the step2 PSUM rotation) are not
        # queued behind step5's tanh+out-DMA block on the serial ACT
        # sequencer (each ACT-queue dma_start costs 667ns of sequencer
        # time); softmax(b+1) then hides under step5(b)+step1(b+2).
        sc = {0: step2(0)}
        sm = {}
        for b in range(BPC):
            sh = softmax_head(b, *sc[b])
            if b + 1 < BPC:
                qws[b + 1] = step1(b + 1)
            sm[b] = softmax(b, *sh)
            e, rdenb = sm[b]

            # --- step 4: c~^T = enc @ e^T (fp16), fold in 1/denom ---
            cn = pcs.tile([P, HT, T], fp16, tag="B2", name=f"cn_{b}")
            for m in range(HT):
                if m % 2 == 0:
                    en = enp.tile([P, ST, 256], fp16, tag="en",
                                  name=f"en_{b}_{m}")
                    nc.sync.dma_start(
                        out=en[:, 0:ks[b], :],
                        in_=encf[b, 0:128 * ks[b], 256 * (m // 2):
                                 256 * (m // 2) + 256]
                        .rearrange("(k p) h -> p k h", p=P))
                hsl = slice(128 * (m % 2), 128 * (m % 2) + 128)
                c_ps = psC.tile([P, T], f32, tag="c", name=f"c_{b}_{m}")
                for k in range(ks[b]):
                    nc.tensor.matmul(c_ps, en[:, k, hsl], e[:, k, :],
                                     start=(k == 0), stop=(k == ks[b] - 1))
                nc.vector.tensor_mul(cn[:, m, :], c_ps, rdenb)

            if b + 1 < BPC:
                prefetch_et(b + 1)
                sc[b + 1] = step2(b + 1)

            # --- step 5: out^T = tanh(WqT @ q^T + WcT @ cn + b), fp16 ---
            for m in range(HT):
                msl = slice(128 * m, 128 * (m + 1))
                if b == BPC - 1 and m == HT - 1:
                    # the very last chunk's tanh->DMA chain is fully
                    # exposed at the end of the kernel: split it into four
                    # T-quarters (each in its own long-dead buffer slot)
                    # so all but the last quarter's chain overlaps
                    # remaining matmuls
                    qpool = [psA, psB, psC, psA]
                    qtag = ["qo", "sc", "c", "qo"]
                    for h4 in range(4):
                        tsl = slice(128 * h4, 128 * (h4 + 1))
                        o_ph = qpool[h4].tile([P, 128], f32, tag=qtag[h4],
                                              name=f"o_{b}_{m}_{h4}")
                        for k in range(HT):
                            nc.tensor.matmul(o_ph, wqf[:, k, msl],
                                             qs[b][0][:, k, tsl],
                                             start=(k == 0), stop=False)
                        for k in range(HT):
                            nc.tensor.matmul(o_ph, wcf[:, k, msl],
                                             cn[:, k, tsl],
                                             start=False, stop=(k == HT - 1))
                        if h4 == 0:
                            oth = otp.tile([P, 128], fp16, tag="ot",
                                           name="oth_a")
                        elif h4 == 1:
                            oth = sp.tile([P, 128], fp16, tag="smax",
                                          name="oth_b")
                        else:
                            oth = fpt.tile([P, 128], fp16, tag="tmp",
                                           name=f"oth_{h4}")
                        nc.scalar.activation(oth, o_ph, AF.Tanh,
                                             bias=bo_sb[:, m:m + 1])
                        nc.sync.dma_start(out=outT[b, msl, tsl], in_=oth)
                    continue
                o_ps = psA.tile([P, T], f32, tag="qo", name=f"o_{b}_{m}")
                for k in range(HT):
                    nc.tensor.matmul(o_ps, wqf[:, k, msl], qs[b][0][:, k, :],
                                     start=(k == 0), stop=False)
                for k in range(HT):
                    nc.tensor.matmul(o_ps, wcf[:, k, msl], cn[:, k, :],
                                     start=False, stop=(k == HT - 1))
                ot = otp.tile([P, T], fp16, tag="ot")
                nc.scalar.activation(ot, o_ps, AF.Tanh, bias=bo_sb[:, m:m + 1])
                # last batch: no input DMAs remain, so out-DMAs use the
                # idle SP queue — the ACT sequencer then only runs tanhs
                # and the final quarter chain is not stuck behind 667ns
                # dma_start sequencer slots
                dq = nc.sync if b == BPC - 1 else nc.scalar
                dq.dma_start(out=outT[b, 128 * m:128 * (m + 1), :],
                             in_=ot)

    nc.compile()
    return nc


def _split16(x):
    """fp16 hi + fp8 pack [hi8, 2^11*lo in fp8] (moving order)."""
    hi = x.astype(np.float16)
    lo = (x - hi.astype(np.float32)) * RSC
    return hi, hi.astype(ml_dtypes.float8_e4m3), lo.astype(ml_dtypes.float8_e4m3)


def kernel(query, encoder_outputs, src_lengths, W_in, W_out, b_out):
    query = np.asarray(query, dtype=np.float32)
    encoder_outputs = np.ascontiguousarray(np.asarray(encoder_outputs, np.float32))
    src_lengths = np.asarray(src_lengths)
    W_in = np.asarray(W_in, dtype=np.float32)
    W_out = np.asarray(W_out, dtype=np.float32)
    b_out = np.asarray(b_out, dtype=np.float32)

    # --- length-sorted batch -> (core, slot) assignment ---
    lens_all = np.asarray(src_lengths, dtype=np.int64)
    order = np.argsort(-lens_all, kind="stable")        # global desc
    assign = np.empty((NCORES, BPC), dtype=np.int64)    # [core, slot] -> batch
    for g in range(BPC):
        for c in range(NCORES):
            assign[c, g] = order[NCORES * g + c]
    ks = tuple(max(2, min(ST, int(
        -(-int(lens_all[order[NCORES * g:NCORES * (g + 1)]].max()) // 128))))
        for g in range(BPC))

    # --- shared (weight) inputs ---
    W_inT = np.ascontiguousarray(W_in.T)                    # [h, o]
    Wihi, Wih8, Wil8 = _split16(W_inT)
    Wi8 = np.ascontiguousarray(np.stack([Wil8, Wih8], axis=0))  # stat: [lo, hi]
    Wqf = np.ascontiguousarray(W_out[:, :H].T).astype(np.float16)
    Wcf = np.ascontiguousarray(W_out[:, H:].T).astype(np.float16)
    bo = np.ascontiguousarray(b_out.reshape(HT, P).T)       # [p, m]

    # --- per-core shards ---
    in_maps = []
    for c in range(NCORES):
        bidx = assign[c]
        q = query[bidx]                                     # [BPC, T, H]
        encs = np.ascontiguousarray(encoder_outputs[bidx])  # [BPC, S, H]
        lens = lens_all[bidx]

        qTa = np.ascontiguousarray(q.transpose(0, 2, 1))    # [BPC, H, T]
        qh, qh8, ql8 = _split16(qTa)
        q8a = np.ascontiguousarray(np.stack([qh8, ql8], axis=1))  # mov: [hi, lo]
        eTa = np.ascontiguousarray(encs.transpose(0, 2, 1))  # [BPC, H, S]
        eh, eh8, el8 = _split16(eTa)
        eT8a = np.ascontiguousarray(np.stack([el8, eh8], axis=1))  # stat: [lo, hi]

        maskca = np.zeros((BPC, P, ST), dtype=np.float32)
        pos = (np.arange(ST)[None, :] * P + np.arange(P)[:, None])  # [P, ST]
        for j in range(BPC):
            maskca[j][pos >= lens[j]] = MASKVAL

        in_maps.append({
            "qhi": qh, "q8": q8a, "eThi": eh, "eT8": eT8a,
            "encf": encs.astype(np.float16),
            "maskc": maskca, "Wihi": Wihi, "Wi8": Wi8,
            "Wqf": Wqf, "Wcf": Wcf, "bo": bo,
        })

    if ks not in _nc_by_ks:
        _nc_by_ks[ks] = _build_nc(ks)
    nc = _nc_by_ks[ks]
    # keep the active nc at index 0 (test.py reads _nc_cache[0])
    if nc in _nc_cache:
        _nc_cache.remove(nc)
    _nc_cache.insert(0, nc)

    res = run_bass_kernel_spmd(nc, in_maps, core_ids=list(range(NCORES)))
    global LAST_RESULTS
    LAST_RESULTS = res

    out = np.empty((B, T, H), dtype=np.float32)
    for c in range(NCORES):
        o = res.results[c]["outT"]                          # [BPC, H, T] fp16
        for g in range(BPC):
            out[assign[c, g]] = o[g].astype(np.float32).T
    return out
